# revision 3
# baseline (speedup 1.0000x reference)
"""Multi-head attention + residual + LayerNorm on 8 TRN2 NeuronCores.

Problem shapes (hardcoded): bs=4, seq=1024, d_model=1024, head=16, dk=64.

Sharding (data parallel over (batch, query-token-half)):
  core c -> batch b = c // 2, query rows [512*(c%2), 512*(c%2)+512).
  Each core computes K/V projections for its batch's full 1024 tokens,
  Q projection for its 512 rows, attention for all 16 heads over its
  512 query rows, and residual+LayerNorm for those rows. No collectives.

Device kernel layout strategy:
  - All matmul inputs are loaded transposed (contraction dim on
    partitions) via bf16 DMA-transpose loads.
  - qp^T [o, qt] and kp^T [o, kt] produced in PSUM, copied to SBUF bf16.
  - vp [kt, o] produced naturally (lhsT = v^T tiles).
  - scores S = qh^T.T @ kh^T in natural [qt, kt] layout; exp on ScalarE
    (scale=1/8 folded, denominator via accum_out); normalize on DVE;
    att written to DRAM bf16.
  - att read back TRANSPOSED via DMA-transpose -> rhs of out^T = vh.T @ att^T
    (PSUM accumulate over kt); PE-transpose of out^T back to natural;
    residual + LayerNorm (bn_stats/bn_aggr) in fp32.
"""

import numpy as np
import ml_dtypes

import concourse.bacc as bacc
import concourse.bass as bass
import concourse.mybir as mybir
import concourse.tile as tile
from concourse.masks import make_identity
from concourse.bass_utils import run_bass_kernel_spmd

P = 128
BS = 4
SEQ = 1024
D = 1024
H = 16
DK = 64
SQ = 512          # query rows per core
SK = SEQ          # kv rows per core
KD = D // P       # 8 contraction tiles
QT = SQ // P      # 4 query-row tiles
KT = SK // P      # 8 kv-row tiles
OT = D // P       # 8 output-feature tiles
EPS = 1e-5

BF = mybir.dt.bfloat16
F32 = mybir.dt.float32
AF = mybir.ActivationFunctionType

N_CORES = 8
BF_NP = ml_dtypes.bfloat16


def _emit(nc):
    """Emit the per-core Tile program."""
    q_bf = nc.dram_tensor("q_bf", (SQ, D), BF, kind="ExternalInput").ap()
    k_bf = nc.dram_tensor("k_bf", (SK, D), BF, kind="ExternalInput").ap()
    v_bf = nc.dram_tensor("v_bf", (SK, D), BF, kind="ExternalInput").ap()
    wq = nc.dram_tensor("wq", (D, D), BF, kind="ExternalInput").ap()
    wk = nc.dram_tensor("wk", (D, D), BF, kind="ExternalInput").ap()
    wv = nc.dram_tensor("wv", (D, D), BF, kind="ExternalInput").ap()
    bq = nc.dram_tensor("bq", (1, D), BF, kind="ExternalInput").ap()
    bk = nc.dram_tensor("bk", (1, D), BF, kind="ExternalInput").ap()
    bv = nc.dram_tensor("bv", (1, D), BF, kind="ExternalInput").ap()
    q_f32 = nc.dram_tensor("q_f32", (SQ, D), F32, kind="ExternalInput").ap()
    gamma = nc.dram_tensor("gamma", (1, D), F32, kind="ExternalInput").ap()
    beta = nc.dram_tensor("beta", (1, D), F32, kind="ExternalInput").ap()

    att_d = nc.dram_tensor("att", (H, SQ, SK), BF, kind="ExternalOutput").ap()
    normed_d = nc.dram_tensor("normed", (SQ, D), F32, kind="ExternalOutput").ap()

    with tile.TileContext(nc) as tc:
        with (
            tc.tile_pool(name="consts", bufs=1) as consts,
            tc.tile_pool(name="proj", bufs=1) as proj,
            tc.tile_pool(name="psum", bufs=2, space="PSUM") as psum,
        ):
            # ---------------- constants ----------------
            ones_row = consts.tile([1, SK], BF)
            nc.vector.memset(ones_row, 1.0)
            ident = consts.tile([P, P], F32)
            make_identity(nc, ident)
            gamma_b = consts.tile([P, D], F32)
            nc.gpsimd.dma_start(out=gamma_b, in_=gamma.to_broadcast((P, D)))
            beta_b = consts.tile([P, D], F32)
            nc.gpsimd.dma_start(out=beta_b, in_=beta.to_broadcast((P, D)))
            eps_t = consts.tile([P, 1], F32)
            nc.vector.memset(eps_t, EPS)
            bq_row = consts.tile([1, D], BF)
            nc.sync.dma_start(out=bq_row, in_=bq)
            bk_row = consts.tile([1, D], BF)
            nc.sync.dma_start(out=bk_row, in_=bk)
            bv_row = consts.tile([1, D], BF)
            nc.sync.dma_start(out=bv_row, in_=bv)

            # ---------------- transposed input loads ----------------
            with tc.tile_pool(name="inputs", bufs=1) as inp:
                wqT = inp.tile([P, KD, D], BF)
                wkT = inp.tile([P, KD, D], BF)
                wvT = inp.tile([P, KD, D], BF)
                kTt = inp.tile([P, KD, SK], BF)
                vTt = inp.tile([P, KD, SK], BF)
                qTt = inp.tile([P, KD, SQ], BF)
                for kd in range(KD):
                    sl = slice(kd * P, (kd + 1) * P)
                    nc.sync.dma_start_transpose(out=wqT[:, kd, :], in_=wq[:, sl])
                    nc.sync.dma_start_transpose(out=wkT[:, kd, :], in_=wk[:, sl])
                    nc.sync.dma_start_transpose(out=wvT[:, kd, :], in_=wv[:, sl])
                    nc.sync.dma_start_transpose(out=kTt[:, kd, :], in_=k_bf[:, sl])
                    nc.sync.dma_start_transpose(out=vTt[:, kd, :], in_=v_bf[:, sl])
                    nc.sync.dma_start_transpose(out=qTt[:, kd, :], in_=q_bf[:, sl])

                q_nat = proj.tile([P, QT, D], F32)
                for t in range(QT):
                    nc.sync.dma_start(
                        out=q_nat[:, t, :], in_=q_f32[t * P : (t + 1) * P, :]
                    )

                # ---------------- projections ----------------
                qpT = proj.tile([P, OT, SQ], BF)  # qp^T[o, qt]
                kpT = proj.tile([P, OT, SK], BF)  # kp^T[o, kt]
                vp = proj.tile([P, KT, D], BF)    # vp[kt, o]

                for i in range(OT):
                    osl = slice(i * P, (i + 1) * P)
                    # qp^T o-tile i: lhsT = Wq^T tile, rhs = q^T
                    ps_q = psum.tile([P, D], F32, tag="wide")
                    for kd in range(KD):
                        nc.tensor.matmul(
                            ps_q[:, :SQ],
                            lhsT=wqT[:, kd, osl],
                            rhs=qTt[:, kd, :],
                            start=(kd == 0),
                            stop=False,
                        )
                    nc.tensor.matmul(
                        ps_q[:, :SQ],
                        lhsT=bq_row[:, osl],
                        rhs=ones_row[:, :SQ],
                        start=False,
                        stop=True,
                    )
                    nc.vector.tensor_copy(out=qpT[:, i, :], in_=ps_q[:, :SQ])

                    # kp^T o-tile i
                    ps_k = psum.tile([P, D], F32, tag="wide")
                    for j in range(2):
                        jsl = slice(j * 512, (j + 1) * 512)
                        for kd in range(KD):
                            nc.tensor.matmul(
                                ps_k[:, jsl],
                                lhsT=wkT[:, kd, osl],
                                rhs=kTt[:, kd, jsl],
                                start=(kd == 0),
                                stop=False,
                            )
                        nc.tensor.matmul(
                            ps_k[:, jsl],
                            lhsT=bk_row[:, osl],
                            rhs=ones_row[:, :512],
                            start=False,
                            stop=True,
                        )
                    nc.vector.tensor_copy(out=kpT[:, i, :], in_=ps_k)

                    # vp kv-row tile i: lhsT = v^T tile, rhs = Wv^T
                    ps_v = psum.tile([P, D], F32, tag="wide")
                    for j in range(2):
                        jsl = slice(j * 512, (j + 1) * 512)
                        for kd in range(KD):
                            nc.tensor.matmul(
                                ps_v[:, jsl],
                                lhsT=vTt[:, kd, osl],
                                rhs=wvT[:, kd, jsl],
                                start=(kd == 0),
                                stop=False,
                            )
                        nc.tensor.matmul(
                            ps_v[:, jsl],
                            lhsT=ones_row[:, :P],
                            rhs=bv_row[:, jsl],
                            start=False,
                            stop=True,
                        )
                    nc.vector.tensor_copy(out=vp[:, i, :], in_=ps_v)

            # ---------------- attention ----------------
            with (
                tc.tile_pool(name="p_sb", bufs=3) as p_pool,
                tc.tile_pool(name="att_sb", bufs=3) as att_pool,
                tc.tile_pool(name="attT", bufs=4) as attT_pool,
                tc.tile_pool(name="den", bufs=8) as den_pool,
                tc.tile_pool(name="outT", bufs=2) as outT_pool,
                tc.tile_pool(name="ln", bufs=2) as ln_pool,
            ):
                out_nat = proj.tile([P, QT, D], F32)

                def emit_softmax(h):
                    po = DK * (h % 2)
                    pr = h // 2
                    for t in range(QT):
                        ps = psum.tile([P, SK], F32, tag="wide")
                        for j in range(2):
                            jsl = slice(j * 512, (j + 1) * 512)
                            nc.tensor.matmul(
                                ps[:, jsl],
                                lhsT=qpT[po : po + DK, pr, t * P : (t + 1) * P],
                                rhs=kpT[po : po + DK, pr, jsl],
                                start=True,
                                stop=True,
                            )
                        p_sb = p_pool.tile([P, SK], BF)
                        den = den_pool.tile([P, 1], F32)
                        nc.scalar.activation(
                            out=p_sb, in_=ps, func=AF.Exp, scale=0.125, accum_out=den
                        )
                        rec = den_pool.tile([P, 1], F32)
                        nc.vector.reciprocal(out=rec, in_=den)
                        att_sb = att_pool.tile([P, SK], BF)
                        nc.vector.tensor_scalar_mul(out=att_sb, in0=p_sb, scalar1=rec)
                        nc.sync.dma_start(
                            out=att_d[h, t * P : (t + 1) * P, :], in_=att_sb
                        )

                def emit_av(h):
                    ps_o = psum.tile([DK, SQ], F32, tag="o")
                    for r in range(KT):
                        attT = attT_pool.tile([P, SQ], BF)
                        nc.sync.dma_start_transpose(
                            out=attT, in_=att_d[h, :, r * P : (r + 1) * P]
                        )
                        nc.tensor.matmul(
                            ps_o,
                            lhsT=vp[:, r, h * DK : (h + 1) * DK],
                            rhs=attT,
                            start=(r == 0),
                            stop=(r == KT - 1),
                        )
                    oT = outT_pool.tile([DK, SQ], F32)
                    nc.vector.tensor_copy(out=oT, in_=ps_o)
                    for t in range(QT):
                        ptr = psum.tile([P, DK], F32, tag="tr")
                        nc.tensor.transpose(
                            ptr, oT[:, t * P : (t + 1) * P], ident[:DK, :DK]
                        )
                        nc.vector.tensor_copy(
                            out=out_nat[:, t, h * DK : (h + 1) * DK], in_=ptr
                        )

                for h in range(H):
                    emit_softmax(h)
                    if h >= 1:
                        emit_av(h - 1)
                emit_av(H - 1)

                # ---------------- residual + LayerNorm ----------------
                for t in range(QT):
                    x = ln_pool.tile([P, D], F32)
                    nc.vector.tensor_add(
                        out=x, in0=q_nat[:, t, :], in1=out_nat[:, t, :]
                    )
                    stats = ln_pool.tile([P, 2, 6], F32)
                    for g in range(2):
                        nc.vector.bn_stats(
                            out=stats[:, g, :], in_=x[:, g * 512 : (g + 1) * 512]
                        )
                    mv = ln_pool.tile([P, 2], F32)
                    nc.vector.bn_aggr(out=mv, in_=stats)
                    std = ln_pool.tile([P, 1], F32)
                    nc.scalar.activation(
                        out=std, in_=mv[:, 1:2], func=AF.Sqrt, bias=eps_t
                    )
                    rstd = ln_pool.tile([P, 1], F32)
                    nc.vector.reciprocal(out=rstd, in_=std)
                    nrm = ln_pool.tile([P, D], F32)
                    nc.vector.tensor_scalar(
                        out=nrm,
                        in0=x,
                        scalar1=mv[:, 0:1],
                        scalar2=rstd,
                        op0=mybir.AluOpType.subtract,
                        op1=mybir.AluOpType.mult,
                    )
                    nc.vector.tensor_mul(out=nrm, in0=nrm, in1=gamma_b)
                    nc.vector.tensor_add(out=nrm, in0=nrm, in1=beta_b)
                    nc.sync.dma_start(
                        out=normed_d[t * P : (t + 1) * P, :], in_=nrm
                    )
    return nc


_NC_CACHE = None


def _get_nc():
    global _NC_CACHE
    if _NC_CACHE is None:
        nc = bacc.Bacc("TRN2", target_bir_lowering=False, debug=False)
        _emit(nc)
        nc.compile()
        _NC_CACHE = nc
    return _NC_CACHE


def _shard_inputs(q, k, v, Wq, bq, Wk, bk, Wv, bv, gamma, beta):
    bf = lambda a: np.ascontiguousarray(a, dtype=np.float32).astype(BF_NP)
    f32 = lambda a: np.ascontiguousarray(a, dtype=np.float32)
    wq_b, wk_b, wv_b = bf(Wq), bf(Wk), bf(Wv)
    bq_b, bk_b, bv_b = (
        bf(bq).reshape(1, D),
        bf(bk).reshape(1, D),
        bf(bv).reshape(1, D),
    )
    gamma_f = f32(gamma).reshape(1, D)
    beta_f = f32(beta).reshape(1, D)
    in_maps = []
    for c in range(N_CORES):
        b = c // 2
        rows = slice((c % 2) * SQ, (c % 2) * SQ + SQ)
        in_maps.append(
            {
                "q_bf": bf(q[b, rows, :]),
                "k_bf": bf(k[b]),
                "v_bf": bf(v[b]),
                "wq": wq_b,
                "wk": wk_b,
                "wv": wv_b,
                "bq": bq_b,
                "bk": bk_b,
                "bv": bv_b,
                "q_f32": f32(q[b, rows, :]),
                "gamma": gamma_f,
                "beta": beta_f,
            }
        )
    return in_maps


def run_sharded(inputs, trace=False, tmpdir=None):
    """Run the SPMD kernel; returns (normed, att_score, BassKernelResults)."""
    assert int(inputs["head"]) == H
    nc = _get_nc()
    in_maps = _shard_inputs(
        inputs["q"], inputs["k"], inputs["v"],
        inputs["Wq"], inputs["bq"], inputs["Wk"], inputs["bk"],
        inputs["Wv"], inputs["bv"], inputs["gamma"], inputs["beta"],
    )
    res = run_bass_kernel_spmd(
        nc, in_maps, core_ids=list(range(N_CORES)), trace=trace, tmpdir=tmpdir
    )
    normed = np.empty((BS, SEQ, D), np.float32)
    att = np.empty((BS, H, SEQ, SK), np.float32)
    for c in range(N_CORES):
        b = c // 2
        rows = slice((c % 2) * SQ, (c % 2) * SQ + SQ)
        out_c = res.results[c]
        normed[b, rows, :] = out_c["normed"]
        att[b, :, rows, :] = np.asarray(out_c["att"]).astype(np.float32)
    return normed, att, res


def kernel(**inputs):
    normed, att, _ = run_sharded(inputs, trace=False)
    return normed, att


# revision 6
# speedup vs baseline: 1.2209x; 1.2209x over previous
"""Multi-head attention + residual + LayerNorm on 8 TRN2 NeuronCores.

Problem shapes (hardcoded): bs=4, seq=1024, d_model=1024, head=16, dk=64.

Sharding (data parallel over (batch, query-token-half)):
  core c -> batch b = c // 2, query rows [512*(c%2), 512*(c%2)+512).
  Each core computes K/V projections for its batch's full 1024 tokens,
  Q projection for its 512 rows, attention for all 16 heads over its
  512 query rows, and residual+LayerNorm for those rows. No collectives.

Device kernel layout strategy:
  - Matmul operands need the contraction dim on partitions, so the host
    feeds q/k/v/W pre-transposed (bf16); all input DMAs are plain
    contiguous loads.
  - qp^T [o, qt] and kp^T [o, kt] produced in PSUM, copied to SBUF bf16.
  - vp [kt, o] produced naturally (lhsT = v^T tiles).
  - scores S = qh^T.T @ kh^T in natural [qt, kt] layout, head pairs
    row-packed on the PE (partitions 0-63 / 64-127); exp on ScalarE
    (scale=1/8 folded, denominator via accum_out); normalize on DVE;
    att written to DRAM bf16 via SWDGE (gpsimd) to keep the HWDGE rings
    free for transposes.
  - att read back TRANSPOSED via DMA-transpose in 4-head groups
    ([2048 x 128] -> [128 x 2048], 32 ops total) -> rhs of
    out^T = vh.T @ att^T (PSUM accumulate over kt); PE-transpose of
    out^T back to natural; residual + LayerNorm (bn_stats) in fp32.
"""

import numpy as np
import ml_dtypes

import concourse.bacc as bacc
import concourse.bass as bass
import concourse.mybir as mybir
import concourse.tile as tile
from concourse.masks import make_identity
from concourse.bass_utils import run_bass_kernel_spmd

P = 128
BS = 4
SEQ = 1024
D = 1024
H = 16
DK = 64
SQ = 512          # query rows per core
SK = SEQ          # kv rows per core
KD = D // P       # 8 contraction tiles
QT = SQ // P      # 4 query-row tiles
KT = SK // P      # 8 kv-row tiles
OT = D // P       # 8 output-feature tiles
HG = 4            # heads per readback group
EPS = 1e-5

BF = mybir.dt.bfloat16
F32 = mybir.dt.float32
AF = mybir.ActivationFunctionType

N_CORES = 8
BF_NP = ml_dtypes.bfloat16


def _emit(nc):
    """Emit the per-core Tile program."""
    qT_d = nc.dram_tensor("qT", (D, SQ), BF, kind="ExternalInput").ap()
    kT_d = nc.dram_tensor("kT", (D, SK), BF, kind="ExternalInput").ap()
    vT_d = nc.dram_tensor("vT", (D, SK), BF, kind="ExternalInput").ap()
    wqT_d = nc.dram_tensor("wqT", (D, D), BF, kind="ExternalInput").ap()
    wkT_d = nc.dram_tensor("wkT", (D, D), BF, kind="ExternalInput").ap()
    wvT_d = nc.dram_tensor("wvT", (D, D), BF, kind="ExternalInput").ap()
    bq = nc.dram_tensor("bq", (1, D), BF, kind="ExternalInput").ap()
    bk = nc.dram_tensor("bk", (1, D), BF, kind="ExternalInput").ap()
    bv = nc.dram_tensor("bv", (1, D), BF, kind="ExternalInput").ap()
    q_f32 = nc.dram_tensor("q_f32", (SQ, D), F32, kind="ExternalInput").ap()
    gamma = nc.dram_tensor("gamma", (1, D), F32, kind="ExternalInput").ap()
    beta = nc.dram_tensor("beta", (1, D), F32, kind="ExternalInput").ap()

    att_d = nc.dram_tensor("att", (H, SQ, SK), BF, kind="ExternalOutput").ap()
    normed_d = nc.dram_tensor("normed", (SQ, D), F32, kind="ExternalOutput").ap()

    def part3(ap):
        # (KD*P, F) dram view -> [p, kd, F] AP for a single big DMA
        return ap.rearrange("(kd p) f -> p kd f", p=P)

    with tile.TileContext(nc) as tc:
        with (
            tc.tile_pool(name="consts", bufs=1) as consts,
            tc.tile_pool(name="proj", bufs=1) as proj,
            tc.tile_pool(name="psum", bufs=2, space="PSUM") as psum,
        ):
            # ---------------- constants ----------------
            ones_row = consts.tile([1, SK], BF)
            nc.vector.memset(ones_row, 1.0)
            ident = consts.tile([P, P], F32)
            make_identity(nc, ident)
            gamma_b = consts.tile([P, D], F32)
            nc.gpsimd.dma_start(out=gamma_b, in_=gamma.to_broadcast((P, D)))
            beta_b = consts.tile([P, D], F32)
            nc.gpsimd.dma_start(out=beta_b, in_=beta.to_broadcast((P, D)))
            eps_t = consts.tile([P, 1], F32)
            nc.vector.memset(eps_t, EPS)
            bq_row = consts.tile([1, D], BF)
            nc.gpsimd.dma_start(out=bq_row, in_=bq)
            bk_row = consts.tile([1, D], BF)
            nc.gpsimd.dma_start(out=bk_row, in_=bk)
            bv_row = consts.tile([1, D], BF)
            nc.gpsimd.dma_start(out=bv_row, in_=bv)

            # ---------------- input loads (pre-transposed on host) ----------
            with tc.tile_pool(name="inputs", bufs=1) as inp:
                wqT = inp.tile([P, KD, D], BF)
                wkT = inp.tile([P, KD, D], BF)
                wvT = inp.tile([P, KD, D], BF)
                kTt = inp.tile([P, KD, SK], BF)
                vTt = inp.tile([P, KD, SK], BF)
                qTt = inp.tile([P, KD, SQ], BF)
                nc.sync.dma_start(out=qTt, in_=part3(qT_d))
                nc.sync.dma_start(out=kTt, in_=part3(kT_d))
                nc.scalar.dma_start(out=vTt, in_=part3(vT_d))
                nc.sync.dma_start(out=wqT, in_=part3(wqT_d))
                nc.scalar.dma_start(out=wkT, in_=part3(wkT_d))
                nc.scalar.dma_start(out=wvT, in_=part3(wvT_d))

                q_nat = proj.tile([P, QT, D], F32)
                for t in range(QT):
                    nc.gpsimd.dma_start(
                        out=q_nat[:, t, :], in_=q_f32[t * P : (t + 1) * P, :]
                    )

                # ---------------- projections ----------------
                qpT = proj.tile([P, OT, SQ], BF)  # qp^T[o, qt]
                kpT = proj.tile([P, OT, SK], BF)  # kp^T[o, kt]
                vp = proj.tile([P, KT, D], BF)    # vp[kt, o]

                for i in range(OT):
                    osl = slice(i * P, (i + 1) * P)
                    # qp^T o-tile
                    ps_q = psum.tile([P, D], F32, tag="wide")
                    for kd in range(KD):
                        nc.tensor.matmul(
                            ps_q[:, :SQ],
                            lhsT=wqT[:, kd, osl],
                            rhs=qTt[:, kd, :],
                            start=(kd == 0),
                            stop=False,
                        )
                    nc.tensor.matmul(
                        ps_q[:, :SQ],
                        lhsT=bq_row[:, osl],
                        rhs=ones_row[:, :SQ],
                        start=False,
                        stop=True,
                    )
                    nc.vector.tensor_copy(out=qpT[:, i, :], in_=ps_q[:, :SQ])

                    # kp^T o-tile
                    ps_k = psum.tile([P, D], F32, tag="wide")
                    for j in range(2):
                        jsl = slice(j * 512, (j + 1) * 512)
                        for kd in range(KD):
                            nc.tensor.matmul(
                                ps_k[:, jsl],
                                lhsT=wkT[:, kd, osl],
                                rhs=kTt[:, kd, jsl],
                                start=(kd == 0),
                                stop=False,
                            )
                        nc.tensor.matmul(
                            ps_k[:, jsl],
                            lhsT=bk_row[:, osl],
                            rhs=ones_row[:, :512],
                            start=False,
                            stop=True,
                        )
                    nc.vector.tensor_copy(out=kpT[:, i, :], in_=ps_k)

                    # vp kv-row tile
                    ps_v = psum.tile([P, D], F32, tag="wide")
                    for j in range(2):
                        jsl = slice(j * 512, (j + 1) * 512)
                        for kd in range(KD):
                            nc.tensor.matmul(
                                ps_v[:, jsl],
                                lhsT=vTt[:, kd, osl],
                                rhs=wvT[:, kd, jsl],
                                start=(kd == 0),
                                stop=False,
                            )
                        nc.tensor.matmul(
                            ps_v[:, jsl],
                            lhsT=ones_row[:, :P],
                            rhs=bv_row[:, jsl],
                            start=False,
                            stop=True,
                        )
                    nc.vector.tensor_copy(out=vp[:, i, :], in_=ps_v)

            # ---------------- attention ----------------
            with (
                tc.tile_pool(name="p_sb", bufs=4) as p_pool,
                tc.tile_pool(name="att_sb", bufs=4) as att_pool,
                tc.tile_pool(name="attT", bufs=3) as attT_pool,
                tc.tile_pool(name="den", bufs=12) as den_pool,
                tc.tile_pool(name="outT", bufs=3) as outT_pool,
                tc.tile_pool(name="ln", bufs=2) as ln_pool,
            ):
                out_nat = proj.tile([P, QT, D], F32)

                def softmax_tail(h, ps):
                    p_sb = p_pool.tile([P, SK], BF)
                    den = den_pool.tile([P, 1], F32)
                    nc.scalar.activation(
                        out=p_sb, in_=ps, func=AF.Exp, scale=0.125, accum_out=den
                    )
                    rec = den_pool.tile([P, 1], F32)
                    nc.vector.reciprocal(out=rec, in_=den)
                    att_sb = att_pool.tile([P, SK], BF)
                    nc.vector.tensor_scalar_mul(out=att_sb, in0=p_sb, scalar1=rec)
                    return att_sb

                def emit_softmax_pair(hp):
                    # heads 2*hp (partitions 0-63) and 2*hp+1 (64-127),
                    # row-packed on the PE via distinct row-groups
                    for t in range(QT):
                        tsl = slice(t * P, (t + 1) * P)
                        ps_a = psum.tile([P, SK], F32, tag="wide")
                        ps_b = psum.tile([P, SK], F32, tag="wide")
                        for j in range(2):
                            jsl = slice(j * 512, (j + 1) * 512)
                            nc.tensor.matmul(
                                ps_a[:, jsl],
                                lhsT=qpT[0:DK, hp, tsl],
                                rhs=kpT[0:DK, hp, jsl],
                                start=True,
                                stop=True,
                            )
                            nc.tensor.matmul(
                                ps_b[:, jsl],
                                lhsT=qpT[DK : 2 * DK, hp, tsl],
                                rhs=kpT[DK : 2 * DK, hp, jsl],
                                start=True,
                                stop=True,
                            )
                        att_a = softmax_tail(2 * hp, ps_a)
                        att_b = softmax_tail(2 * hp + 1, ps_b)
                        nc.gpsimd.dma_start(out=att_d[2 * hp, tsl, :], in_=att_a)
                        nc.gpsimd.dma_start(out=att_d[2 * hp + 1, tsl, :], in_=att_b)

                def emit_av_group(g):
                    # heads [HG*g, HG*(g+1)): grouped transposed readback
                    ps_os = [
                        psum.tile([DK, SQ], F32, tag="o", bufs=HG, name=f"ps_o{u}")
                        for u in range(HG)
                    ]
                    for r in range(KT):
                        rsl = slice(r * P, (r + 1) * P)
                        attT = attT_pool.tile([P, HG * SQ], BF)
                        src = att_d[HG * g : HG * (g + 1), :, rsl].rearrange(
                            "h q k -> (h q) k"
                        )
                        nc.sync.dma_start_transpose(out=attT, in_=src)
                        for u in range(HG):
                            h = HG * g + u
                            nc.tensor.matmul(
                                ps_os[u],
                                lhsT=vp[:, r, h * DK : (h + 1) * DK],
                                rhs=attT[:, u * SQ : (u + 1) * SQ],
                                start=(r == 0),
                                stop=(r == KT - 1),
                            )
                    for u in range(HG):
                        h = HG * g + u
                        oT = outT_pool.tile([DK, SQ], F32)
                        nc.vector.tensor_copy(out=oT, in_=ps_os[u])
                        for t in range(QT):
                            ptr = psum.tile([P, DK], F32, tag="o", bufs=HG)
                            nc.tensor.transpose(
                                ptr, oT[:, t * P : (t + 1) * P], ident[:DK, :DK]
                            )
                            nc.vector.tensor_copy(
                                out=out_nat[:, t, h * DK : (h + 1) * DK], in_=ptr
                            )

                n_groups = H // HG
                pairs_per_group = HG // 2
                for g in range(n_groups):
                    for hp in range(pairs_per_group * g, pairs_per_group * (g + 1)):
                        emit_softmax_pair(hp)
                    if g >= 1:
                        emit_av_group(g - 1)
                emit_av_group(n_groups - 1)

                # ---------------- residual + LayerNorm ----------------
                for t in range(QT):
                    x = ln_pool.tile([P, D], F32)
                    nc.vector.tensor_add(
                        out=x, in0=q_nat[:, t, :], in1=out_nat[:, t, :]
                    )
                    stats = ln_pool.tile([P, 2, 6], F32)
                    for g in range(2):
                        nc.vector.bn_stats(
                            out=stats[:, g, :], in_=x[:, g * 512 : (g + 1) * 512]
                        )
                    mv = ln_pool.tile([P, 2], F32)
                    nc.vector.bn_aggr(out=mv, in_=stats)
                    std = ln_pool.tile([P, 1], F32)
                    nc.scalar.activation(
                        out=std, in_=mv[:, 1:2], func=AF.Sqrt, bias=eps_t
                    )
                    rstd = ln_pool.tile([P, 1], F32)
                    nc.vector.reciprocal(out=rstd, in_=std)
                    nrm = ln_pool.tile([P, D], F32)
                    nc.vector.tensor_scalar(
                        out=nrm,
                        in0=x,
                        scalar1=mv[:, 0:1],
                        scalar2=rstd,
                        op0=mybir.AluOpType.subtract,
                        op1=mybir.AluOpType.mult,
                    )
                    nc.vector.tensor_mul(out=nrm, in0=nrm, in1=gamma_b)
                    nc.vector.tensor_add(out=nrm, in0=nrm, in1=beta_b)
                    nc.gpsimd.dma_start(
                        out=normed_d[t * P : (t + 1) * P, :], in_=nrm
                    )
    return nc


_NC_CACHE = None


def _get_nc():
    global _NC_CACHE
    if _NC_CACHE is None:
        nc = bacc.Bacc("TRN2", target_bir_lowering=False, debug=False)
        _emit(nc)
        nc.compile()
        _NC_CACHE = nc
    return _NC_CACHE


def _shard_inputs(q, k, v, Wq, bq, Wk, bk, Wv, bv, gamma, beta):
    bf = lambda a: np.ascontiguousarray(np.asarray(a, dtype=np.float32)).astype(BF_NP)
    bfT = lambda a: np.ascontiguousarray(
        np.asarray(a, dtype=np.float32).T.astype(BF_NP)
    )
    f32 = lambda a: np.ascontiguousarray(np.asarray(a, dtype=np.float32))
    wqT, wkT, wvT = bfT(Wq), bfT(Wk), bfT(Wv)
    bq_b, bk_b, bv_b = (
        bf(bq).reshape(1, D),
        bf(bk).reshape(1, D),
        bf(bv).reshape(1, D),
    )
    gamma_f = f32(gamma).reshape(1, D)
    beta_f = f32(beta).reshape(1, D)
    kT = [bfT(k[b]) for b in range(BS)]
    vT = [bfT(v[b]) for b in range(BS)]
    in_maps = []
    for c in range(N_CORES):
        b = c // 2
        rows = slice((c % 2) * SQ, (c % 2) * SQ + SQ)
        in_maps.append(
            {
                "qT": bfT(q[b, rows, :]),
                "kT": kT[b],
                "vT": vT[b],
                "wqT": wqT,
                "wkT": wkT,
                "wvT": wvT,
                "bq": bq_b,
                "bk": bk_b,
                "bv": bv_b,
                "q_f32": f32(q[b, rows, :]),
                "gamma": gamma_f,
                "beta": beta_f,
            }
        )
    return in_maps


def run_sharded(inputs, trace=False, tmpdir=None):
    """Run the SPMD kernel; returns (normed, att_score, BassKernelResults)."""
    assert int(inputs["head"]) == H
    nc = _get_nc()
    in_maps = _shard_inputs(
        inputs["q"], inputs["k"], inputs["v"],
        inputs["Wq"], inputs["bq"], inputs["Wk"], inputs["bk"],
        inputs["Wv"], inputs["bv"], inputs["gamma"], inputs["beta"],
    )
    res = run_bass_kernel_spmd(
        nc, in_maps, core_ids=list(range(N_CORES)), trace=trace, tmpdir=tmpdir
    )
    normed = np.empty((BS, SEQ, D), np.float32)
    att = np.empty((BS, H, SEQ, SK), np.float32)
    for c in range(N_CORES):
        b = c // 2
        rows = slice((c % 2) * SQ, (c % 2) * SQ + SQ)
        out_c = res.results[c]
        normed[b, rows, :] = out_c["normed"]
        att[b, :, rows, :] = np.asarray(out_c["att"]).astype(np.float32)
    return normed, att, res


def kernel(**inputs):
    normed, att, _ = run_sharded(inputs, trace=False)
    return normed, att


# revision 11
# speedup vs baseline: 1.4269x; 1.1687x over previous
"""Multi-head attention + residual + LayerNorm on 8 TRN2 NeuronCores.

Problem shapes (hardcoded): bs=4, seq=1024, d_model=1024, head=16, dk=64.

Sharding (data parallel over (batch, query-token-half)):
  core c -> batch b = c // 2, query rows [512*(c%2), 512*(c%2)+512).
  Each core computes K/V projections for its batch's full 1024 tokens,
  Q projection for its 512 rows, attention for all 16 heads over its
  512 query rows, and residual+LayerNorm for those rows. No collectives.

Device kernel structure (single fused pipeline):
  - Host feeds q/k/v/W pre-transposed bf16 (contraction dim on
    partitions); all input DMAs are plain contiguous loads, ordered so
    the q/k projections can start as early as possible.
  - Main loop interleaves projection o-tiles with score+softmax work for
    the previous head pair, so ScalarE's exp stream hides under the
    PE's projection matmuls and the PE never idles long enough to lose
    the HAM clock boost.
  - Biases are fused into the PSUM->SBUF copies on the DVE (per-
    partition tensor_scalar for qp^T/kp^T, broadcast tensor_tensor for
    vp) - no PE bias matmuls.
  - scores S = qh^T.T @ kh^T in natural [qt, kt] layout, head pairs
    row-packed on the PE (partitions 0-63 / 64-127); exp on ScalarE
    (scale=1/8 folded, denominator via accum_out); normalize on DVE;
    att written to DRAM bf16 via SWDGE (gpsimd).
  - att read back TRANSPOSED via DMA-transpose in 4-head groups
    ([2048 x 128] -> [128 x 2048], 32 ops, sync ring only), issued as
    soon as each group's att is written so the readbacks overlap the
    main loop; the att.V matmuls run as one dense batch at the end,
    PE-transposed back to natural, then residual + LayerNorm (fp32).
"""

import numpy as np
import ml_dtypes

import concourse.bacc as bacc
import concourse.bass as bass
import concourse.mybir as mybir
import concourse.tile as tile
from concourse.masks import make_identity
from concourse.bass_utils import run_bass_kernel_spmd

P = 128
BS = 4
SEQ = 1024
D = 1024
H = 16
DK = 64
SQ = 512          # query rows per core
SK = SEQ          # kv rows per core
KD = D // P       # 8 contraction tiles
QT = SQ // P      # 4 query-row tiles
KT = SK // P      # 8 kv-row tiles
OT = D // P       # 8 output-feature tiles
HG = 4            # heads per readback group
EPS = 1e-5

BF = mybir.dt.bfloat16
F32 = mybir.dt.float32
AF = mybir.ActivationFunctionType

N_CORES = 8
BF_NP = ml_dtypes.bfloat16


def _emit(nc):
    """Emit the per-core Tile program."""
    qT_d = nc.dram_tensor("qT", (D, SQ), BF, kind="ExternalInput").ap()
    kT_d = nc.dram_tensor("kT", (D, SK), BF, kind="ExternalInput").ap()
    vT_d = nc.dram_tensor("vT", (D, SK), BF, kind="ExternalInput").ap()
    wqT_d = nc.dram_tensor("wqT", (D, D), BF, kind="ExternalInput").ap()
    wkT_d = nc.dram_tensor("wkT", (D, D), BF, kind="ExternalInput").ap()
    wvT_d = nc.dram_tensor("wvT", (D, D), BF, kind="ExternalInput").ap()
    bq = nc.dram_tensor("bq", (1, D), F32, kind="ExternalInput").ap()
    bk = nc.dram_tensor("bk", (1, D), F32, kind="ExternalInput").ap()
    bv = nc.dram_tensor("bv", (1, D), F32, kind="ExternalInput").ap()
    q_bf = nc.dram_tensor("q_bf", (SQ, D), BF, kind="ExternalInput").ap()
    gamma = nc.dram_tensor("gamma", (1, D), F32, kind="ExternalInput").ap()
    beta = nc.dram_tensor("beta", (1, D), F32, kind="ExternalInput").ap()

    att_d = nc.dram_tensor("att", (H, SQ, SK), BF, kind="ExternalOutput").ap()
    normed_d = nc.dram_tensor("normed", (SQ, D), F32, kind="ExternalOutput").ap()

    def part3(ap):
        # (KD*P, F) dram view -> [p, kd, F] AP for a single big DMA
        return ap.rearrange("(kd p) f -> p kd f", p=P)

    with tile.TileContext(nc) as tc:
        with (
            tc.tile_pool(name="consts", bufs=1) as consts,
            tc.tile_pool(name="proj", bufs=1) as proj,
            tc.tile_pool(name="psum", bufs=2, space="PSUM") as psum,
        ):
            # ---------------- constants (gpsimd queue, tiny) ----------------
            ident = consts.tile([P, P], F32)
            make_identity(nc, ident)
            gamma_b = consts.tile([P, D], F32)
            nc.gpsimd.dma_start(out=gamma_b, in_=gamma.to_broadcast((P, D)))
            beta_b = consts.tile([P, D], F32)
            nc.gpsimd.dma_start(out=beta_b, in_=beta.to_broadcast((P, D)))
            bv_bc = consts.tile([P, D], BF)
            nc.gpsimd.dma_start(out=bv_bc, in_=bv.to_broadcast((P, D)))
            eps_t = consts.tile([P, 1], F32)
            nc.vector.memset(eps_t, EPS)
            # per-partition bias columns: bq_col[p, i] = bq[i*128 + p]
            bq_col = consts.tile([P, KD], F32)
            nc.gpsimd.dma_start(out=bq_col, in_=bq.rearrange("a (i p) -> p (a i)", p=P))
            bk_col = consts.tile([P, KD], F32)
            nc.gpsimd.dma_start(out=bk_col, in_=bk.rearrange("a (i p) -> p (a i)", p=P))

            # ---------------- input loads (pre-transposed on host) ----------
            # ordered so qp/kp (and the first score matmuls) start earliest
            with tc.tile_pool(name="inputs", bufs=1) as inp:
                wqT = inp.tile([P, KD, D], BF)
                wkT = inp.tile([P, KD, D], BF)
                wvT = inp.tile([P, KD, D], BF)
                kTt = inp.tile([P, KD, SK], BF)
                vTt = inp.tile([P, KD, SK], BF)
                qTt = inp.tile([P, KD, SQ], BF)
                nc.sync.dma_start(out=qTt, in_=part3(qT_d))
                nc.scalar.dma_start(out=wqT, in_=part3(wqT_d))
                nc.sync.dma_start(out=kTt, in_=part3(kT_d))
                nc.scalar.dma_start(out=wkT, in_=part3(wkT_d))
                nc.sync.dma_start(out=vTt, in_=part3(vT_d))
                nc.scalar.dma_start(out=wvT, in_=part3(wvT_d))

                qpT = proj.tile([P, OT, SQ], BF)  # qp^T[o, qt]
                kpT = proj.tile([P, OT, SK], BF)  # kp^T[o, kt]
                vp = proj.tile([P, KT, D], BF)    # vp[kt, o]
                out_nat = proj.tile([P, QT, D], F32)

                with (
                    tc.tile_pool(name="p_sb", bufs=3) as p_pool,
                    tc.tile_pool(name="att_sb", bufs=3) as att_pool,
                    tc.tile_pool(name="attT", bufs=7) as attT_pool,
                    tc.tile_pool(name="den", bufs=12) as den_pool,
                    tc.tile_pool(name="outT", bufs=2) as outT_pool,
                    tc.tile_pool(name="ln", bufs=1) as ln_pool,
                ):
                    def emit_qp(i):
                        osl = slice(i * P, (i + 1) * P)
                        ps_q = psum.tile([P, D], F32, tag="wide", name="ps_q")
                        for kd in range(KD):
                            nc.tensor.matmul(
                                ps_q[:, :SQ],
                                lhsT=wqT[:, kd, osl],
                                rhs=qTt[:, kd, :],
                                start=(kd == 0),
                                stop=(kd == KD - 1),
                            )
                        nc.vector.tensor_scalar_add(
                            out=qpT[:, i, :],
                            in0=ps_q[:, :SQ],
                            scalar1=bq_col[:, i : i + 1],
                        )

                    def emit_kp(i):
                        osl = slice(i * P, (i + 1) * P)
                        ps_k = psum.tile([P, D], F32, tag="wide", name="ps_k")
                        for j in range(2):
                            jsl = slice(j * 512, (j + 1) * 512)
                            for kd in range(KD):
                                nc.tensor.matmul(
                                    ps_k[:, jsl],
                                    lhsT=wkT[:, kd, osl],
                                    rhs=kTt[:, kd, jsl],
                                    start=(kd == 0),
                                    stop=(kd == KD - 1),
                                )
                        nc.vector.tensor_scalar_add(
                            out=kpT[:, i, :],
                            in0=ps_k,
                            scalar1=bk_col[:, i : i + 1],
                        )

                    def emit_vp(i):
                        osl = slice(i * P, (i + 1) * P)
                        ps_v = psum.tile([P, D], F32, tag="wide", name="ps_v")
                        for j in range(2):
                            jsl = slice(j * 512, (j + 1) * 512)
                            for kd in range(KD):
                                nc.tensor.matmul(
                                    ps_v[:, jsl],
                                    lhsT=vTt[:, kd, osl],
                                    rhs=wvT[:, kd, jsl],
                                    start=(kd == 0),
                                    stop=(kd == KD - 1),
                                )
                        nc.vector.tensor_tensor(
                            out=vp[:, i, :],
                            in0=ps_v,
                            in1=bv_bc,
                            op=mybir.AluOpType.add,
                        )

                    def softmax_tail(h, ps):
                        p_sb = p_pool.tile([P, SK], BF, name="p_sb")
                        den = den_pool.tile([P, 1], F32, name="den")
                        nc.scalar.activation(
                            out=p_sb, in_=ps, func=AF.Exp, scale=0.125, accum_out=den
                        )
                        rec = den_pool.tile([P, 1], F32, name="rec")
                        nc.vector.reciprocal(out=rec, in_=den)
                        att_sb = att_pool.tile([P, SK], BF, name="att_sb")
                        nc.vector.tensor_scalar_mul(out=att_sb, in0=p_sb, scalar1=rec)
                        return att_sb

                    def emit_softmax_pair(hp):
                        # heads 2*hp (partitions 0-63) and 2*hp+1 (64-127)
                        for t in range(QT):
                            tsl = slice(t * P, (t + 1) * P)
                            ps_a = psum.tile([P, SK], F32, tag="wide", name="ps_a")
                            ps_b = psum.tile([P, SK], F32, tag="wide", name="ps_b")
                            for j in range(2):
                                jsl = slice(j * 512, (j + 1) * 512)
                                nc.tensor.matmul(
                                    ps_a[:, jsl],
                                    lhsT=qpT[0:DK, hp, tsl],
                                    rhs=kpT[0:DK, hp, jsl],
                                    start=True,
                                    stop=True,
                                )
                                nc.tensor.matmul(
                                    ps_b[:, jsl],
                                    lhsT=qpT[DK : 2 * DK, hp, tsl],
                                    rhs=kpT[DK : 2 * DK, hp, jsl],
                                    start=True,
                                    stop=True,
                                )
                            att_a = softmax_tail(2 * hp, ps_a)
                            att_b = softmax_tail(2 * hp + 1, ps_b)
                            nc.gpsimd.dma_start(out=att_d[2 * hp, tsl, :], in_=att_a)
                            nc.gpsimd.dma_start(
                                out=att_d[2 * hp + 1, tsl, :], in_=att_b
                            )

                    attT_tiles = {}

                    def emit_readback(g):
                        for r in range(KT):
                            rsl = slice(r * P, (r + 1) * P)
                            attT = attT_pool.tile([P, HG * SQ], BF, name="attT")
                            src = att_d[HG * g : HG * (g + 1), :, rsl].rearrange(
                                "h q k -> (h q) k"
                            )
                            nc.sync.dma_start_transpose(out=attT, in_=src)
                            attT_tiles[(g, r)] = attT

                    def emit_av_group(g):
                        ps_os = [
                            psum.tile([DK, SQ], F32, tag="o", bufs=HG, name=f"ps_o{u}")
                            for u in range(HG)
                        ]
                        for r in range(KT):
                            attT = attT_tiles.pop((g, r))
                            for u in range(HG):
                                h = HG * g + u
                                nc.tensor.matmul(
                                    ps_os[u],
                                    lhsT=vp[:, r, h * DK : (h + 1) * DK],
                                    rhs=attT[:, u * SQ : (u + 1) * SQ],
                                    start=(r == 0),
                                    stop=(r == KT - 1),
                                )
                        for u in range(HG):
                            h = HG * g + u
                            oT = outT_pool.tile([DK, SQ], F32, name="oT")
                            nc.vector.tensor_copy(out=oT, in_=ps_os[u])
                            for t in range(QT):
                                ptr = psum.tile(
                                    [P, DK], F32, tag="o", bufs=HG, name="ptr"
                                )
                                nc.tensor.transpose(
                                    ptr, oT[:, t * P : (t + 1) * P], ident[:DK, :DK]
                                )
                                nc.vector.tensor_copy(
                                    out=out_nat[:, t, h * DK : (h + 1) * DK], in_=ptr
                                )

                    # ---------- fused main loop ----------
                    for i in range(OT):
                        emit_qp(i)
                        emit_kp(i)
                        emit_vp(i)
                        if i >= 1:
                            emit_softmax_pair(i - 1)
                        # pairs 2g and 2g+1 are done after softmax pair 2g+1,
                        # i.e. after the i = 2g+2 iteration's softmax
                        if i >= 3 and i % 2 == 1:
                            emit_readback((i - 3) // 2)
                    emit_softmax_pair(OT - 1)
                    emit_readback(H // HG - 1)

                    for g in range(H // HG):
                        emit_av_group(g)

                    # ---------------- residual + LayerNorm ----------------
                    for t in range(QT):
                        qres = ln_pool.tile([P, D], BF, name="qres")
                        nc.gpsimd.dma_start(
                            out=qres, in_=q_bf[t * P : (t + 1) * P, :]
                        )
                        x = out_nat[:, t, :]
                        nc.vector.tensor_add(out=x, in0=x, in1=qres)
                        stats = ln_pool.tile([P, 2, 6], F32, name="stats")
                        for g in range(2):
                            nc.vector.bn_stats(
                                out=stats[:, g, :], in_=x[:, g * 512 : (g + 1) * 512]
                            )
                        mv = ln_pool.tile([P, 2], F32, name="mv")
                        nc.vector.bn_aggr(out=mv, in_=stats)
                        std = ln_pool.tile([P, 1], F32, name="std")
                        nc.scalar.activation(
                            out=std, in_=mv[:, 1:2], func=AF.Sqrt, bias=eps_t
                        )
                        rstd = ln_pool.tile([P, 1], F32, name="rstd")
                        nc.vector.reciprocal(out=rstd, in_=std)
                        nrm = ln_pool.tile([P, D], F32, name="nrm")
                        nc.vector.tensor_scalar(
                            out=nrm,
                            in0=x,
                            scalar1=mv[:, 0:1],
                            scalar2=rstd,
                            op0=mybir.AluOpType.subtract,
                            op1=mybir.AluOpType.mult,
                        )
                        nc.vector.tensor_mul(out=nrm, in0=nrm, in1=gamma_b)
                        nc.vector.tensor_add(out=nrm, in0=nrm, in1=beta_b)
                        nc.gpsimd.dma_start(
                            out=normed_d[t * P : (t + 1) * P, :], in_=nrm
                        )
    return nc


_NC_CACHE = None


def _get_nc():
    global _NC_CACHE
    if _NC_CACHE is None:
        nc = bacc.Bacc("TRN2", target_bir_lowering=False, debug=False)
        _emit(nc)
        nc.compile()
        _NC_CACHE = nc
    return _NC_CACHE


def _shard_inputs(q, k, v, Wq, bq, Wk, bk, Wv, bv, gamma, beta):
    bfT = lambda a: np.ascontiguousarray(
        np.asarray(a, dtype=np.float32).T.astype(BF_NP)
    )
    f32 = lambda a: np.ascontiguousarray(np.asarray(a, dtype=np.float32))
    wqT, wkT, wvT = bfT(Wq), bfT(Wk), bfT(Wv)
    bq_f, bk_f, bv_f = (
        f32(bq).reshape(1, D),
        f32(bk).reshape(1, D),
        f32(bv).reshape(1, D),
    )
    gamma_f = f32(gamma).reshape(1, D)
    beta_f = f32(beta).reshape(1, D)
    kT = [bfT(k[b]) for b in range(BS)]
    vT = [bfT(v[b]) for b in range(BS)]
    in_maps = []
    for c in range(N_CORES):
        b = c // 2
        rows = slice((c % 2) * SQ, (c % 2) * SQ + SQ)
        in_maps.append(
            {
                "qT": bfT(q[b, rows, :]),
                "kT": kT[b],
                "vT": vT[b],
                "wqT": wqT,
                "wkT": wkT,
                "wvT": wvT,
                "bq": bq_f,
                "bk": bk_f,
                "bv": bv_f,
                "q_bf": f32(q[b, rows, :]).astype(BF_NP),
                "gamma": gamma_f,
                "beta": beta_f,
            }
        )
    return in_maps


def run_sharded(inputs, trace=False, tmpdir=None):
    """Run the SPMD kernel; returns (normed, att_score, BassKernelResults)."""
    assert int(inputs["head"]) == H
    nc = _get_nc()
    in_maps = _shard_inputs(
        inputs["q"], inputs["k"], inputs["v"],
        inputs["Wq"], inputs["bq"], inputs["Wk"], inputs["bk"],
        inputs["Wv"], inputs["bv"], inputs["gamma"], inputs["beta"],
    )
    res = run_bass_kernel_spmd(
        nc, in_maps, core_ids=list(range(N_CORES)), trace=trace, tmpdir=tmpdir
    )
    normed = np.empty((BS, SEQ, D), np.float32)
    att = np.empty((BS, H, SEQ, SK), np.float32)
    for c in range(N_CORES):
        b = c // 2
        rows = slice((c % 2) * SQ, (c % 2) * SQ + SQ)
        out_c = res.results[c]
        normed[b, rows, :] = out_c["normed"]
        att[b, :, rows, :] = np.asarray(out_c["att"]).astype(np.float32)
    return normed, att, res


def kernel(**inputs):
    normed, att, _ = run_sharded(inputs, trace=False)
    return normed, att
